# revision 10
# baseline (speedup 1.0000x reference)
"""CenterLossLayer Trainium2 kernel (8-core SPMD, Bass/Tile).

Strategy: shard by LABEL RANGE (12500 classes per core). Host sorts samples
by label (pure index manipulation), packs them into 128-slot tiles such that
no class straddles a tile boundary, and hands each core its samples in
sorted order. All arithmetic (center gather, counts, deltas, segment sums,
center update, loss) happens on device:

  per tile of 128 sorted samples:
    sel[p,j]  = (label_p == label_j)             (DVE is_equal vs PE transpose)
    n_p       = sum_j sel[p,j]                   (exact per-class count)
    d0        = centers[label] - features        (rows via dma_gather)
    loss_p    = sum_d d0^2                       (ACT Square + accum)
    comb      = sel @ d0                         (PE f32 matmul, exact group sums)
    value_p   = -ALPHA * comb_p / (1 + n_p)
  dma_scatter_add adds value rows into new_centers (pre-initialized with
  centers). First-occurrence slots target their class row (globally unique
  -> no RMW races); all other slots target discarded dump rows.

kernel(**inputs) takes FULL inputs, returns (result[B,1], new_centers[C,D]).
"""

import sys

sys.path.insert(0, "/opt/trn_rl_repo")

import numpy as np

from concourse import bass, bacc, mybir
import concourse.tile as tile
from concourse.bass_utils import run_bass_kernel_spmd

ALPHA = 0.5
NUM_CLASSES = 100000
FEAT_DIM = 128
BATCH = 131072
NCORES = 8

P = 128


class Cfg:
    def __init__(self, num_classes, batch, ncores, s_pad, dump_rows=1536):
        assert num_classes % ncores == 0
        self.C = num_classes
        self.B = batch
        self.ncores = ncores
        self.csh = num_classes // ncores  # classes per core
        assert s_pad % 2048 == 0
        self.s_pad = s_pad              # padded slots per core
        self.n_tiles = s_pad // P       # tiles per core
        self.dump = dump_rows           # discard rows appended to the table
        self.tbl = self.csh + dump_rows # per-core table rows
        self.n_chunks = s_pad // 2048   # dma chunks (16 tiles each)


FULL_CFG = Cfg(NUM_CLASSES, BATCH, NCORES, s_pad=18432)


# ----------------------------------------------------------------------------
# device program
# ----------------------------------------------------------------------------

def build_program(cfg: Cfg, ablate: frozenset = frozenset()):
    nc = bacc.Bacc("TRN2", target_bir_lowering=False, debug=False,
                   num_devices=cfg.ncores)
    f32 = mybir.dt.float32
    i16 = mybir.dt.int16
    T = cfg.n_tiles
    TPC = 16  # tiles per dma chunk
    idx_cols = cfg.s_pad // 16

    feat = nc.declare_dram_parameter("feat", [P, T * P], f32, isOutput=False)
    ctr = nc.declare_dram_parameter("ctr", [cfg.tbl, P], f32, isOutput=False)
    gidx = nc.declare_dram_parameter("gidx", [P, idx_cols], i16, isOutput=False)
    sidx = nc.declare_dram_parameter("sidx", [P, idx_cols], i16, isOutput=False)
    labT = nc.declare_dram_parameter("labT", [P, T], f32, isOutput=False)
    ident_in = nc.declare_dram_parameter("ident", [P, P], f32, isOutput=False)
    nctr = nc.declare_dram_parameter("nctr", [cfg.tbl, P], f32, isOutput=True)
    lout = nc.declare_dram_parameter("lout", [P, T], f32, isOutput=True)

    with tile.TileContext(nc) as tc:
        with (
            tc.tile_pool(name="const", bufs=1) as cpool,
            tc.tile_pool(name="io", bufs=3) as iopool,
            tc.tile_pool(name="work", bufs=3) as wpool,
            tc.tile_pool(name="sct", bufs=2) as spool,
            tc.tile_pool(name="ps_t", bufs=3, space="PSUM") as pst,
            tc.tile_pool(name="ps_m", bufs=3, space="PSUM") as psm,
        ):
            # new_centers := centers  (dense init; scatter-adds come later)
            nc.sync.dma_start(out=nctr[:], in_=ctr[:])

            ident = cpool.tile([P, P], f32)
            nc.sync.dma_start(out=ident[:], in_=ident_in[:])
            lab = cpool.tile([P, T], f32)
            nc.sync.dma_start(out=lab[:], in_=labT[:])
            gix = cpool.tile([P, idx_cols], i16)
            nc.sync.dma_start(out=gix[:], in_=gidx[:])
            six = cpool.tile([P, idx_cols], i16)
            nc.sync.dma_start(out=six[:], in_=sidx[:])
            losbuf = cpool.tile([P, T], f32)

            GPT = 8  # tiles per dma_gather call (1024 idxs; >=2048 overflows
            #          the SWDGE descriptor ring on HW)
            for kc in range(cfg.n_chunks):
                sct = spool.tile([P, TPC, P], f32, tag="sct")
                fk = iopool.tile([P, TPC, P], f32, tag="fk")
                nc.sync.dma_start(
                    out=fk[:], in_=feat[:, kc * TPC * P : (kc + 1) * TPC * P]
                )
                ck = iopool.tile([P, TPC, P], f32, tag="ck")
                for tt in range(0, TPC, GPT):
                    nc.gpsimd.dma_gather(
                        out_ap=ck[:, tt : tt + GPT, :],
                        in_ap=ctr[:],
                        idxs_ap=gix[:, (kc * TPC + tt) * 8 : (kc * TPC + tt + GPT) * 8],
                        num_idxs=GPT * P,
                        num_idxs_reg=GPT * P,
                        elem_size=P,
                    )
                if True:
                    for t in range(TPC):
                        gt = kc * TPC + t        # global tile id
                        lt = t                   # tile id within scatter group
                        # label row broadcast: PE transpose of the label column
                        ltp = pst.tile([P, P], f32, space="PSUM", tag="ltp")
                        nc.tensor.transpose(
                            out=ltp[:],
                            in_=lab[:, gt : gt + 1].to_broadcast([P, P]),
                            identity=ident[:],
                        )
                        sel = wpool.tile([P, P], f32, tag="sel")
                        nc.vector.tensor_tensor(
                            out=sel[:],
                            in0=lab[:, gt : gt + 1].to_broadcast([P, P]),
                            in1=ltp[:],
                            op=mybir.AluOpType.is_equal,
                        )
                        # d0 = centers_row - feature
                        d0 = wpool.tile([P, P], f32, tag="d0")
                        nc.vector.tensor_tensor(
                            out=d0[:],
                            in0=ck[:, t, :],
                            in1=fk[:, t, :],
                            op=mybir.AluOpType.subtract,
                        )
                        # loss = sum(d0^2) along free dim (ACT square+accum)
                        sq = wpool.tile([P, P], f32, tag="sq")
                        nc.scalar.activation(
                            out=sq[:],
                            in_=d0[:],
                            func=mybir.ActivationFunctionType.Square,
                            accum_out=losbuf[:, gt : gt + 1],
                        )
                        # group sums: comb = sel @ d0  (sel symmetric)
                        mm = psm.tile([P, P], f32, space="PSUM", tag="mm")
                        nc.tensor.matmul(
                            out=mm[:], lhsT=sel[:], rhs=d0[:], start=True, stop=True
                        )
                        # n = row-sum(sel); rv = 1/(1+n)
                        ncol = wpool.tile([P, 1], f32, tag="ncol")
                        nc.vector.tensor_reduce(
                            out=ncol[:], in_=sel[:],
                            axis=mybir.AxisListType.X, op=mybir.AluOpType.add,
                        )
                        n1 = wpool.tile([P, 1], f32, tag="n1")
                        nc.vector.tensor_scalar(
                            out=n1[:], in0=ncol[:], scalar1=1.0, scalar2=None,
                            op0=mybir.AluOpType.add,
                        )
                        rv = wpool.tile([P, 1], f32, tag="rv")
                        if "recip" in ablate:
                            nc.vector.memset(rv[:], 0.5)
                        else:
                            nc.vector.reciprocal(out=rv[:], in_=n1[:])
                        # scatter value rows: -ALPHA * comb * rv
                        if "stt" in ablate:
                            nc.vector.tensor_scalar(
                                out=sct[:, lt, :], in0=mm[:], scalar1=-ALPHA,
                                scalar2=None, op0=mybir.AluOpType.mult,
                            )
                        else:
                            nc.vector.scalar_tensor_tensor(
                                out=sct[:, lt, :],
                                in0=mm[:],
                                scalar=-ALPHA,
                                in1=rv[:].to_broadcast([P, P]),
                                op0=mybir.AluOpType.mult,
                                op1=mybir.AluOpType.mult,
                            )
                nc.gpsimd.dma_scatter_add(
                    out_ap=nctr[:],
                    in_ap=sct[:],
                    idxs_ap=six[:, kc * TPC * 8 : (kc + 1) * TPC * 8],
                    num_idxs=TPC * P,
                    num_idxs_reg=TPC * P,
                    elem_size=P,
                )
            nc.sync.dma_start(out=lout[:], in_=losbuf[:])
    nc.finalize()
    return nc


# ----------------------------------------------------------------------------
# host sharding / packing
# ----------------------------------------------------------------------------

def host_pack(labels: np.ndarray, cfg: Cfg):
    """Sort by label, range-shard, pack into straddle-free 128-slot tiles.

    Returns per-core dicts of index arrays (no feature data movement here).
    """
    labels = np.asarray(labels).reshape(-1).astype(np.int64)
    B = labels.shape[0]
    order = np.argsort(labels, kind="stable")
    slab = labels[order]
    bounds = np.searchsorted(slab, np.arange(cfg.ncores + 1) * cfg.csh)
    cores = []
    for c in range(cfg.ncores):
        lo, hi = bounds[c], bounds[c + 1]
        samp = order[lo:hi]            # original sample idx, sorted by label
        lab = slab[lo:hi] - c * cfg.csh  # local labels, ascending
        n = lab.shape[0]
        # pack runs into tiles of 128 without straddling
        starts = np.flatnonzero(np.r_[True, lab[1:] != lab[:-1]])
        lens = np.diff(np.r_[starts, n])
        assert lens.max(initial=0) <= P, "class run exceeds one tile"
        slot = np.empty(n, np.int64)
        cur = 0
        for s, L in zip(starts.tolist(), lens.tolist()):
            room = P - (cur % P)
            if L > room:
                cur += room
            slot[s : s + L] = np.arange(cur, cur + L)
            cur += L
        assert cur <= cfg.s_pad, f"core {c}: {cur} slots > s_pad {cfg.s_pad}"

        samp_at = np.full(cfg.s_pad, -1, np.int64)
        samp_at[slot] = samp
        real = samp_at >= 0

        lab_at = np.empty(cfg.s_pad, np.float64)
        sl = np.arange(cfg.s_pad)
        lab_at[:] = -1.0 - (sl % P)      # pads: distinct within tile
        lab_at[slot] = lab

        gidx = np.zeros(cfg.s_pad, np.int16)
        gidx[slot] = lab.astype(np.int16)

        first = np.zeros(cfg.s_pad, bool)
        first[slot[starts]] = True
        sct = (cfg.csh + (sl % cfg.dump)).astype(np.int16)
        sct[slot[starts]] = lab[starts].astype(np.int16)

        cores.append(
            dict(samp_at=samp_at, real=real, lab_at=lab_at.astype(np.float32),
                 gidx=gidx, sct=sct)
        )
    return cores


def _wrap_idx(a: np.ndarray) -> np.ndarray:
    """[S] int16 -> [128, S/16] wrapped layout replicated to 8 groups."""
    w = a.reshape(-1, 16).T  # [16, S/16]
    return np.tile(w, (8, 1)).copy()


def make_in_maps(features, centers, cores, cfg: Cfg):
    features = np.asarray(features, dtype=np.float32)
    centers = np.asarray(centers, dtype=np.float32)
    T = cfg.n_tiles
    in_maps = []
    ident = np.eye(P, dtype=np.float32)
    for c, m in enumerate(cores):
        fs = np.zeros((cfg.s_pad, P), np.float32)
        fs[m["real"]] = features[m["samp_at"][m["real"]]]
        feat_sw = np.ascontiguousarray(
            fs.reshape(T, P, P).transpose(1, 0, 2).reshape(P, T * P)
        )
        ctab = np.zeros((cfg.tbl, P), np.float32)
        ctab[: cfg.csh] = centers[c * cfg.csh : (c + 1) * cfg.csh]
        in_maps.append(
            {
                "feat": feat_sw,
                "ctr": ctab,
                "gidx": _wrap_idx(m["gidx"]),
                "sidx": _wrap_idx(m["sct"]),
                "labT": np.ascontiguousarray(m["lab_at"].reshape(T, P).T),
                "ident": ident,
            }
        )
    return in_maps


def unshard(results, cores, cfg: Cfg):
    result = np.empty((cfg.B, 1), np.float32)
    new_centers = np.empty((cfg.C, P), np.float32)
    for c, (res, m) in enumerate(zip(results, cores)):
        new_centers[c * cfg.csh : (c + 1) * cfg.csh] = res["nctr"][: cfg.csh]
        loss_sorted = res["lout"].T.reshape(cfg.s_pad)  # slot i = [i%128, i//128]
        real = m["real"]
        result[m["samp_at"][real], 0] = loss_sorted[real]
    return result, new_centers


# ----------------------------------------------------------------------------
# entry point
# ----------------------------------------------------------------------------

_NC_CACHE = {}


def _get_nc(cfg: Cfg):
    key = (cfg.C, cfg.B, cfg.s_pad)
    if key not in _NC_CACHE:
        _NC_CACHE[key] = build_program(cfg)
    return _NC_CACHE[key]


def run(features, labels, centers, cfg: Cfg, **spmd_kwargs):
    cores = host_pack(labels, cfg)
    in_maps = make_in_maps(features, centers, cores, cfg)
    nc = _get_nc(cfg)
    br = run_bass_kernel_spmd(nc, in_maps, list(range(cfg.ncores)), **spmd_kwargs)
    result, new_centers = unshard(br.results, cores, cfg)
    return result, new_centers, br


def kernel(features, labels, centers):
    result, new_centers, _ = run(features, labels, centers, FULL_CFG)
    return result, new_centers
